# revision 36
# baseline (speedup 1.0000x reference)
"""Trainium2 Bass kernel for nn_CLFormer (3-block linear-attention transformer).

Sharding: pure data parallel — batch 32 split as 4 per NeuronCore across 8
cores; all parameters replicated; outputs concatenated.

Per-core layout: 4 batches x 32 channels packed onto the 128 SBUF partitions
("channel-major" [128=4bx32c, L]). The kv-gram and k-softmax denominator
contract over tokens, so a token-major copy is needed per block: block 0
reads host-packed token-major chunked bf16 tiles of x (with a baked-in ones
column per 128-token chunk so the gram matmul also accumulates ksum);
blocks 1-2 round-trip h through DRAM and use the XBAR DMA transpose (which
writes contiguous chunks only — their ksum row comes from ones-stationary
matmuls into spare columns of the G PSUM bank). Attention-out + FC1 fuse
into M1 = diag(1/ksum)@KV@W1, held block-diagonally [128,128] so one matmul
serves all 4 batches; W2 likewise. All weights/biases arrive pre-replicated
in two host-packed tensors (2 DMAs instead of ~84 small ones).
"""
import sys
import numpy as np

for _p in ("/opt/trn_rl_repo", "/root/.axon_site/_ro/trn_rl_repo"):
    if _p not in sys.path:
        sys.path.append(_p)

from contextlib import ExitStack

import concourse.bass as bass
import concourse.mybir as mybir
import bass_rust
from concourse import tile
from concourse.masks import make_identity
from concourse.bass_utils import run_bass_kernel_spmd

F32 = mybir.dt.float32
BF16 = mybir.dt.bfloat16
AF = mybir.ActivationFunctionType
MUL = mybir.AluOpType.mult
ADD = mybir.AluOpType.add

P = 128
B_LOC = 4            # batches per core
C = 32               # channels
L = 16384            # sequence length
NB = 3               # transformer blocks
DOUT = 10
HEADS = 4
DH = 8
BN_EPS = 1e-5
DEBUG = False

SLC = 2048           # slice width (tokens per pipeline slice)
NSL = L // SLC       # 8 slices
NCH = SLC // 128     # 16 chunks per slice
EXT = 129            # chunk width incl. the ones column (block 0)
WIN = 1024           # phase-2 z2/gelu2 window width
NW = L // WIN        # 16 windows

NPBF = NB * C + NB * P        # packed bf16 param cols: W1 x3, W2bd x3
NPF = 2 * NB + C + DOUT + 6   # packed f32 param cols


# ---------------------------------------------------------------- waitfix --
_WF_SKIP = {"InstEventSemaphore"}
_wf_ctr = [0]


def _fix_sync_waits(nc):
    """Hoist excess sync waits onto InstEventSemaphore (this walrus build
    accepts only 1 wait per instruction). The event-sem executes on the same
    engine stream immediately before, preserving semantics."""
    for fn in nc.m.functions:
        new_blocks = []
        for blk in fn.blocks:
            out = []
            for ins in blk.instructions:
                tname = type(ins).__name__
                si = ins.sync_info
                if si is None or tname in _WF_SKIP:
                    out.append(ins)
                    continue
                waits = list(si.on_wait)
                if len(waits) <= 1:
                    out.append(ins)
                    continue
                keep = waits[-1:]
                excess = waits[:-1]
                for i in range(0, len(excess), 2):
                    chunk = excess[i:i + 2]
                    _wf_ctr[0] += 1
                    ev = mybir.InstEventSemaphore(
                        name=f"wfix{_wf_ctr[0]}", ins=[], outs=[])
                    ev.engine = ins.engine
                    ev.sync_info = mybir.SyncInfo(on_wait=chunk, on_update=[])
                    out.append(ev)
                ins.sync_info = mybir.SyncInfo(
                    on_wait=keep, on_update=list(si.on_update))
                out.append(ins)
            nb = bass_rust.BasicBlock(name=blk.name, instructions=out)
            new_blocks.append(nb)
        fn.blocks = new_blocks


# ---------------------------------------------------------------- program --
def build_program(reps=1):
    nc = bass.Bass()

    x_d = nc.declare_dram_parameter("x", [NSL, P, NCH * EXT], BF16, isOutput=False)
    pbf_d = nc.declare_dram_parameter("pbf", [P, NPBF], BF16, isOutput=False)
    pf_d = nc.declare_dram_parameter("pf32", [P, NPF], F32, isOutput=False)
    out_d = nc.declare_dram_parameter("out", [B_LOC, DOUT], F32, isOutput=True)
    if DEBUG:
        dbgG_d = nc.declare_dram_parameter("dbg_G", [P, EXT], F32, isOutput=True)
        dbgM1_d = nc.declare_dram_parameter("dbg_M1", [P, P], F32, isOutput=True)

    with ExitStack() as ctx:
        tc = ctx.enter_context(tile.TileContext(nc))
        cst = ctx.enter_context(tc.tile_pool(name="cst", bufs=1))
        hex_ = ctx.enter_context(tc.tile_pool(name="hex", bufs=3))
        etm = ctx.enter_context(tc.tile_pool(name="etm", bufs=3))
        qtm = ctx.enter_context(tc.tile_pool(name="qtm", bufs=2))
        sqp = ctx.enter_context(tc.tile_pool(name="sqp", bufs=3))
        bigq = ctx.enter_context(tc.tile_pool(name="bigq", bufs=2))
        hcm = ctx.enter_context(tc.tile_pool(name="hcm", bufs=3))
        a1p = ctx.enter_context(tc.tile_pool(name="a1p", bufs=3))
        smal = ctx.enter_context(tc.tile_pool(name="smal", bufs=2))
        gps = ctx.enter_context(tc.tile_pool(name="gps", bufs=1, space="PSUM"))
        z1p = ctx.enter_context(tc.tile_pool(name="z1p", bufs=2, space="PSUM"))
        z2p = ctx.enter_context(tc.tile_pool(name="z2p", bufs=1, space="PSUM"))
        qps = ctx.enter_context(tc.tile_pool(name="qps", bufs=1, space="PSUM"))
        drp = ctx.enter_context(tc.tile_pool(name="drp", bufs=8, space="DRAM"))

        # ---- params (two host-packed tensors, one DMA each) -------------
        pbf = cst.tile([P, NPBF], BF16)
        nc.sync.dma_start(pbf[:], pbf_d[:])
        pf = cst.tile([P, NPF], F32)
        nc.sync.dma_start(pf[:], pf_d[:])
        W1rep = [pbf[:, C * i:C * (i + 1)] for i in range(NB)]
        W2bd = [pbf[:, NB * C + P * i:NB * C + P * (i + 1)] for i in range(NB)]
        b1rep = [pf[:, i:i + 1] for i in range(NB)]
        b2rep = [pf[:, NB + i:NB + i + 1] for i in range(NB)]
        o = 2 * NB
        Whrep = pf[:, o:o + C]
        Wfrep = pf[:, o + C:o + C + DOUT]
        o += C + DOUT
        bh_r = pf[:, o:o + 1]
        bng_r = pf[:, o + 1:o + 2]
        bnb_r = pf[:, o + 2:o + 3]
        bnm_r = pf[:, o + 3:o + 4]
        bnv_r = pf[:, o + 4:o + 5]
        bf_s = pf[:, o + 5:o + 6]

        # ---- constants --------------------------------------------------
        ident = cst.tile([P, P], BF16)
        make_identity(nc, ident[:])
        ones_bf = cst.tile([P, 1], BF16)
        nc.vector.memset(ones_bf[:], 1.0)
        id11 = cst.tile([1, 1], F32)
        nc.vector.memset(id11[:], 1.0)
        headmask = cst.tile([P, P], BF16)
        nc.vector.memset(headmask[:], 1.0)
        hm_v = headmask[:].rearrange("p (g i) -> p g i", i=DH)
        nc.gpsimd.affine_select(
            out=hm_v, in_=hm_v, pattern=[[-DH, P // DH], [0, DH]],
            compare_op=mybir.AluOpType.is_ge, fill=0.0,
            base=0, channel_multiplier=1)
        nc.gpsimd.affine_select(
            out=hm_v, in_=hm_v, pattern=[[DH, P // DH], [0, DH]],
            compare_op=mybir.AluOpType.is_ge, fill=0.0,
            base=DH - 1, channel_multiplier=-1)

        # BN eval folding: y_bn = y_raw * svecL + tvec, where y_raw is the
        # UNSCALED pooled-sum matmul output (missing bias and the 1/L mean).
        eps_t = cst.tile([P, 1], F32)
        nc.vector.memset(eps_t[:], BN_EPS)
        sq_t = cst.tile([P, 1], F32)
        nc.scalar.activation(sq_t[:], bnv_r, AF.Sqrt, bias=eps_t[:])
        rs_t = cst.tile([P, 1], F32)
        nc.vector.reciprocal(rs_t[:], sq_t[:])
        svec = cst.tile([P, 1], F32)
        nc.vector.tensor_tensor(svec[:], rs_t[:], bng_r, op=MUL)
        svecL = cst.tile([P, 1], F32)
        nc.vector.tensor_scalar_mul(svecL[:], svec[:], 1.0 / L)
        t0 = cst.tile([P, 1], F32)
        nc.vector.tensor_tensor(t0[:], bh_r, bnm_r,
                                op=mybir.AluOpType.subtract)
        t1 = cst.tile([P, 1], F32)
        nc.vector.tensor_tensor(t1[:], t0[:], svec[:], op=MUL)
        tvec = cst.tile([P, 1], F32)
        nc.vector.tensor_tensor(tvec[:], t1[:], bnb_r, op=ADD)

        # (repetition loop for benchmarking only; reps=1 in production)
        for _rep in range(reps):
            pooled_parts = cst.tile([P, NW], F32, tag=f"pool_{_rep}")

            def emit_exp(blk, s, he):
                """E = exp(h) for one token-major slice (Act engine)."""
                ext = EXT if blk == 0 else 128
                et = etm.tile([P, NCH * ext], BF16, tag="etm")
                nc.scalar.activation(et[:], he[:], AF.Exp)
                return et

            def emit_rest(blk, s, he, et, G_ps, q_cm):
                """Everything after exp for one slice: gram/ksum (PE),
                softmax denominator (DVE), q multiply (GPSIMD), q transpose
                (PE) + copies (DVE)."""
                ext = EXT if blk == 0 else 128
                # gram: G[d, e] += E_chunk.T @ h_chunk; block 0's ones
                # column also accumulates ksum into G[:, 128].
                for c in range(NCH):
                    nc.tensor.matmul(
                        G_ps[:, 0:ext],
                        et[:, ext * c:ext * c + 128],
                        he[:, ext * c:ext * c + ext],
                        start=(s == 0 and c == 0),
                        stop=(s == NSL - 1 and c == NCH - 1),
                    )
                if blk > 0:
                    # ksum row, folded by 256: [1,256] += ones.T @ E
                    for c in range(NCH // 2):
                        nc.tensor.matmul(
                            G_ps[0:1, 128:384],
                            ones_bf[:],
                            et[:, 256 * c:256 * (c + 1)],
                            start=(s == 0 and c == 0),
                            stop=(s == NSL - 1 and c == NCH // 2 - 1),
                        )
                # q-softmax denominator: segmented sum over d (free dim)
                et_v = et[:].rearrange("p (c e) -> p c e", e=ext)
                et_dv = et_v[:, :, 0:128].rearrange(
                    "p c (g d) -> p c g d", d=DH)
                sq = sqp.tile([P, NCH * 16], F32, tag="sq")
                nc.vector.reduce_sum(
                    sq[:].rearrange("p (c g) -> p c g", g=16),
                    et_dv,
                    axis=mybir.AxisListType.X,
                )
                rq = sqp.tile([P, NCH * 16], F32, tag="rq")
                nc.vector.reciprocal(rq[:], sq[:])
                # q = E * (1/sq) broadcast over d  (GPSIMD)
                qt = qtm.tile([P, SLC], BF16, tag="qtm")
                nc.gpsimd.tensor_tensor(
                    qt[:].rearrange("p (c g d) -> p c g d", g=16, d=DH),
                    et_dv,
                    rq[:].rearrange("p (c g) -> p c g", g=16)
                        .unsqueeze(-1).broadcast_to([P, NCH, 16, DH]),
                    op=MUL,
                )
                # transpose q to channel-major via PE
                for g in range(NCH // 8):
                    qp = qps.tile([P, 1024], BF16, tag="qp")
                    for k in range(8):
                        c = 8 * g + k
                        nc.tensor.transpose(
                            qp[:, 128 * k:128 * (k + 1)],
                            qt[:, 128 * c:128 * (c + 1)],
                            ident[:],
                        )
                    nc.vector.tensor_copy(
                        q_cm[:, SLC * s + 1024 * g: SLC * s + 1024 * (g + 1)],
                        qp[:],
                    )

            # ---- block 0 phase 1 (standalone, from host-packed x) ---------
            G_ps = gps.tile([P, 384], F32, tag="G")
            q_cm = bigq.tile([P, L], BF16, tag="qcm")
            for s in range(NSL):
                he = hex_.tile([P, NCH * EXT], BF16, tag="hex")
                nc.sync.dma_start(he[:], x_d[s])
                et = etm.tile([P, NCH * EXT], BF16, tag="etm")
                nc.scalar.activation(et[:], he[:], AF.Exp)
                emit_rest(0, s, he, et, G_ps, q_cm)

            for blk in range(NB):
                # ============================ M1 build ======================
                # M1 = diag(1/ksum) @ G_m @ W1, written block-diagonally
                # into [128, 128]. ksum: block 0 from G_ps[:, 128] (ones
                # column); blocks 1-2 from the folded ksum row, transposed.
                ksC = smal.tile([P, 1], F32, tag="ksC")
                if blk == 0:
                    nc.vector.reciprocal(ksC[:], G_ps[:, 128:EXT])
                else:
                    ksr_sb = smal.tile([1, P], F32, tag="ksr_sb")
                    nc.vector.reduce_sum(
                        ksr_sb[:],
                        G_ps[0:1, 128:384].rearrange("p (a k) -> p k a", a=2),
                        axis=mybir.AxisListType.X,
                    )
                    kT_ps = gps.tile([P, C], F32, tag="G")
                    nc.tensor.transpose(kT_ps[:, 0:1], ksr_sb[:], id11[:])
                    nc.vector.reciprocal(ksC[:], kT_ps[:, 0:1])
                G_sb = smal.tile([P, P], BF16, tag="Gsb")
                nc.vector.tensor_tensor(G_sb[:], G_ps[:, 0:128], headmask[:], op=MUL)
                # per-batch 32x32 diag-block transpose on the DVE stream
                # transposer (the off-diag blocks transpose too; unread)
                Gt = smal.tile([P, P], BF16, tag="gt2sb")
                nc.vector.transpose(Gt[:], G_sb[:])
                M1u_ps = gps.tile([P, P], F32, tag="G")
                for b in range(B_LOC):
                    sl = slice(C * b, C * (b + 1))
                    nc.tensor.matmul(
                        M1u_ps[sl, C * b:C * (b + 1)], Gt[sl, C * b:C * (b + 1)],
                        W1rep[blk][sl, :],
                        tile_position=(C * b, C * b),
                    )
                if DEBUG and blk == 0:
                    dG = smal.tile([P, EXT], F32, tag="dG")
                    nc.vector.tensor_copy(dG[:], G_ps[:, 0:EXT])
                    nc.sync.dma_start(dbgG_d[:], dG[:])
                M1 = smal.tile([P, P], BF16, tag="m1")
                nc.vector.memset(M1[:], 0.0)
                for b in range(B_LOC):
                    sl = slice(C * b, C * (b + 1))
                    nc.vector.tensor_scalar_mul(
                        M1[sl, C * b:C * (b + 1)],
                        M1u_ps[sl, C * b:C * (b + 1)], ksC[sl, :])
                if DEBUG and blk == 0:
                    dM = smal.tile([P, P], F32, tag="dM")
                    nc.vector.tensor_copy(dM[:], M1[:])
                    nc.sync.dma_start(dbgM1_d[:], dM[:])

                # ============================ phase 2 (channel-major) =======
                # Software-pipelined two ways: z1/gelu1 run two windows ahead
                # of z2/gelu2, AND the NEXT block's phase 1 is interleaved
                # into this block's window loop (the engines execute their
                # queues in program order, so without this the Act engine
                # would run all gelus before any next-block exp). Exps are
                # batched (EXP_AT) to amortize the exp<->gelu activation
                # table swaps; the per-slice gram/q work (REST_AT) is spread
                # one slice per window to fit the PE queue's slack.
                last = blk == NB - 1
                EXP_AT = {7: (0, 1, 2), 13: (3, 4, 5), 15: (6,)}
                REST_AT = {9: (0,), 11: (1,), 13: (2,), 15: (3,)}
                new_h_dr = []
                he_n = [None] * NSL
                et_n = [None] * NSL
                a1t = [None] * 3
                hn = None
                if not last:
                    G_next = gps.tile([P, 384], F32, tag="G")
                    q_next = bigq.tile([P, L], BF16, tag="qcm")

                def emit_z1(v):
                    z1 = z1p.tile([P, WIN], F32, tag="z1")
                    for j in range(2):
                        nc.tensor.matmul(
                            z1[:, 512 * j:512 * (j + 1)], M1[:],
                            q_cm[:, WIN * v + 512 * j:WIN * v + 512 * (j + 1)])
                    a1 = a1p.tile([P, WIN], BF16, tag="a1")
                    nc.scalar.activation(a1[:], z1[:], AF.Gelu, bias=b1rep[blk])
                    a1t[v % 3] = a1

                for v in range(2):
                    emit_z1(v)
                for w in range(NW):
                    z2 = z2p.tile([P, WIN], F32, tag="z2")
                    for j in range(2):
                        nc.tensor.matmul(
                            z2[:, 512 * j:512 * (j + 1)], W2bd[blk],
                            a1t[w % 3][:, 512 * j:512 * (j + 1)],
                        )
                    if w % 2 == 0:
                        hn = hcm.tile([P, SLC], BF16, tag="hcm")
                    dst = hn[:, WIN * (w % 2):WIN * (w % 2 + 1)]
                    if last:
                        nc.scalar.activation(
                            dst, z2[:], AF.Gelu, bias=b2rep[blk],
                            accum_out=pooled_parts[:, w:w + 1],
                        )
                    else:
                        nc.scalar.activation(
                            dst, z2[:], AF.Gelu, bias=b2rep[blk])
                    if w % 2 == 1 and not last:
                        s = w // 2
                        hdr = drp.tile([P, SLC], BF16, tag="hdr")
                        nc.sync.dma_start(hdr[:], hn[:])
                        new_h_dr.append(hdr)
                        he = hex_.tile([P, NCH * 128], BF16, tag="hex")
                        nc.sync.dma_start_transpose(
                            out=he[:].rearrange("p (c l) -> p c l", l=128),
                            in_=hdr[:],
                        )
                        he_n[s] = he
                    if w + 2 < NW:
                        emit_z1(w + 2)
                    if not last:
                        for s in EXP_AT.get(w, ()):
                            et_n[s] = emit_exp(blk + 1, s, he_n[s])
                        for s in REST_AT.get(w, ()):
                            emit_rest(blk + 1, s, he_n[s], et_n[s],
                                      G_next, q_next)
                if not last:
                    et_n[7] = emit_exp(blk + 1, 7, he_n[7])
                    for s in (4, 5, 6, 7):
                        emit_rest(blk + 1, s, he_n[s], et_n[s],
                                  G_next, q_next)
                    G_ps, q_cm = G_next, q_next

            # ============================ head ==============================
            psum_ = smal.tile([P, 1], F32, tag="poolsum")
            nc.vector.reduce_sum(psum_[:], pooled_parts[:],
                                 axis=mybir.AxisListType.X)
            y_ps = gps.tile([P, 32], F32, tag="G")
            for b in range(B_LOC):
                sl = slice(C * b, C * (b + 1))
                nc.tensor.matmul(
                    y_ps[sl, 0:1], Whrep[sl, :], psum_[sl, :],
                    tile_position=(C * b, C * b),
                )
            ybn = smal.tile([P, 1], F32, tag="ybn")
            nc.vector.tensor_scalar(
                ybn[:], y_ps[:, 0:1], svecL[:], tvec[:], op0=MUL, op1=ADD,
            )
            yg = smal.tile([P, 1], F32, tag="yg")
            nc.scalar.activation(yg[:], ybn[:], AF.Gelu)
            o_ps = gps.tile([P, 32], F32, tag="G")
            for b in range(B_LOC):
                nc.tensor.matmul(
                    o_ps[C * b:C * b + DOUT, 0:1],
                    Wfrep[C * b:C * (b + 1), :],
                    yg[C * b:C * (b + 1), :],
                    tile_position=(C * b, C * b),
                )
            ob = smal.tile([P, 1], F32, tag="ob")
            for b in range(B_LOC):
                sl = slice(C * b, C * b + DOUT)
                nc.vector.tensor_tensor(ob[sl, :], o_ps[sl, 0:1], bf_s[sl, :], op=ADD)
            for b in range(B_LOC):
                nc.sync.dma_start(
                    out_d[b, :], ob[C * b:C * b + DOUT, 0],
                )

    _fix_sync_waits(nc)
    return nc


def _pack_x(xc):
    """[B_LOC, C, L] f32 -> [NSL, 128, NCH*129] bf16 token-major chunked
    tiles with a ones column per 128-token chunk (gram ksum extension)."""
    import ml_dtypes
    xt = np.ascontiguousarray(xc.transpose(2, 0, 1)).reshape(L, P)
    xs = xt.reshape(NSL, NCH, 128, P).transpose(0, 2, 1, 3)  # [s, t, k, col]
    out = np.empty((NSL, 128, NCH, EXT), dtype=ml_dtypes.bfloat16)
    out[..., 0:128] = xs.astype(ml_dtypes.bfloat16)
    out[..., 128] = 1.0
    return out.reshape(NSL, 128, NCH * EXT)


def _pack_params(arrs):
    """Host-side pre-replication of all weights/biases into one bf16 and one
    f32 [128, K] tensor (see build_program for the column layout)."""
    import ml_dtypes
    reps = []
    for i in range(NB):
        reps.append(np.tile(arrs["fcW1"][i], (B_LOC, 1)))          # [128,32]
    eye4 = np.eye(B_LOC, dtype=np.float32)
    for i in range(NB):
        reps.append(np.kron(eye4, arrs["fcW2"][i]))                # [128,128]
    pbf = np.concatenate(reps, axis=1).astype(ml_dtypes.bfloat16)

    cols = []
    for i in range(NB):
        cols.append(np.tile(arrs["fcb1"][i], B_LOC)[:, None])
    for i in range(NB):
        cols.append(np.tile(arrs["fcb2"][i], B_LOC)[:, None])
    cols.append(np.tile(arrs["Wh"], (B_LOC, 1)))                   # [128,32]
    cols.append(np.tile(arrs["Wf"], (B_LOC, 1)))                   # [128,10]
    for k in ("bh", "bn_gamma", "bn_beta", "bn_mean", "bn_var"):
        cols.append(np.tile(arrs[k], B_LOC)[:, None])
    bf_col = np.zeros((P, 1), np.float32)
    for b in range(B_LOC):
        bf_col[C * b:C * b + DOUT, 0] = arrs["bf"]
    cols.append(bf_col)
    pf = np.concatenate(cols, axis=1).astype(np.float32)
    assert pf.shape[1] == NPF, pf.shape
    return pbf, pf


def prep_in_maps(arrs, n_cores=8):
    x = arrs["x"]
    bl = x.shape[0] // n_cores
    pbf, pf = _pack_params(arrs)
    return [
        {"x": _pack_x(x[bl * i: bl * (i + 1)]), "pbf": pbf, "pf32": pf}
        for i in range(n_cores)
    ]


_NC_CACHE = [None]


def kernel(**inputs) -> np.ndarray:
    arrs = {k: np.asarray(v, dtype=np.float32) for k, v in inputs.items()}
    n_cores = 8

    if _NC_CACHE[0] is None:
        _NC_CACHE[0] = build_program()
    nc = _NC_CACHE[0]

    in_maps = prep_in_maps(arrs, n_cores)
    res = run_bass_kernel_spmd(nc, in_maps, list(range(n_cores))).results
    return np.concatenate([res[i]["out"] for i in range(n_cores)], axis=0)


# revision 39
# speedup vs baseline: 1.0021x; 1.0021x over previous
"""Trainium2 Bass kernel for nn_CLFormer (3-block linear-attention transformer).

Sharding: pure data parallel — batch 32 split as 4 per NeuronCore across 8
cores; all parameters replicated; outputs concatenated.

Per-core layout: 4 batches x 32 channels packed onto the 128 SBUF partitions
("channel-major" [128=4bx32c, L]). The kv-gram and k-softmax denominator
contract over tokens, so a token-major copy is needed per block: block 0
reads host-packed token-major chunked bf16 tiles of x (with a baked-in ones
column per 128-token chunk so the gram matmul also accumulates ksum);
blocks 1-2 round-trip h through DRAM and use the XBAR DMA transpose (which
writes contiguous chunks only — their ksum row comes from ones-stationary
matmuls into spare columns of the G PSUM bank). Attention-out + FC1 fuse
into M1 = diag(1/ksum)@KV@W1, held block-diagonally [128,128] so one matmul
serves all 4 batches; W2 likewise. All weights/biases arrive pre-replicated
in two host-packed tensors (2 DMAs instead of ~84 small ones).
"""
import sys
import numpy as np

for _p in ("/opt/trn_rl_repo", "/root/.axon_site/_ro/trn_rl_repo"):
    if _p not in sys.path:
        sys.path.append(_p)

from contextlib import ExitStack

import concourse.bass as bass
import concourse.mybir as mybir
import bass_rust
from concourse import tile
from concourse.masks import make_identity
from concourse.bass_utils import run_bass_kernel_spmd

F32 = mybir.dt.float32
BF16 = mybir.dt.bfloat16
AF = mybir.ActivationFunctionType
MUL = mybir.AluOpType.mult
ADD = mybir.AluOpType.add

P = 128
B_LOC = 4            # batches per core
C = 32               # channels
L = 16384            # sequence length
NB = 3               # transformer blocks
DOUT = 10
HEADS = 4
DH = 8
BN_EPS = 1e-5
DEBUG = False

SLC = 2048           # slice width (tokens per pipeline slice)
NSL = L // SLC       # 8 slices
NCH = SLC // 128     # 16 chunks per slice
EXT = 129            # chunk width incl. the ones column (block 0)
WIN = 1024           # phase-2 z2/gelu2 window width
NW = L // WIN        # 16 windows

NPBF = NB * C + NB * P        # packed bf16 param cols: W1 x3, W2bd x3
NPF = 2 * NB + C + DOUT + 6   # packed f32 param cols


# ---------------------------------------------------------------- waitfix --
_WF_SKIP = {"InstEventSemaphore"}
_wf_ctr = [0]


def _fix_sync_waits(nc):
    """Hoist excess sync waits onto InstEventSemaphore (this walrus build
    accepts only 1 wait per instruction). The event-sem executes on the same
    engine stream immediately before, preserving semantics."""
    for fn in nc.m.functions:
        new_blocks = []
        for blk in fn.blocks:
            out = []
            for ins in blk.instructions:
                tname = type(ins).__name__
                si = ins.sync_info
                if si is None or tname in _WF_SKIP:
                    out.append(ins)
                    continue
                waits = list(si.on_wait)
                if len(waits) <= 1:
                    out.append(ins)
                    continue
                keep = waits[-1:]
                excess = waits[:-1]
                for i in range(0, len(excess), 2):
                    chunk = excess[i:i + 2]
                    _wf_ctr[0] += 1
                    ev = mybir.InstEventSemaphore(
                        name=f"wfix{_wf_ctr[0]}", ins=[], outs=[])
                    ev.engine = ins.engine
                    ev.sync_info = mybir.SyncInfo(on_wait=chunk, on_update=[])
                    out.append(ev)
                ins.sync_info = mybir.SyncInfo(
                    on_wait=keep, on_update=list(si.on_update))
                out.append(ins)
            nb = bass_rust.BasicBlock(name=blk.name, instructions=out)
            new_blocks.append(nb)
        fn.blocks = new_blocks


# ---------------------------------------------------------------- program --
def build_program(reps=1):
    nc = bass.Bass()

    x_d = nc.declare_dram_parameter("x", [NSL, P, NCH * EXT], BF16, isOutput=False)
    pbf_d = nc.declare_dram_parameter("pbf", [P, NPBF], BF16, isOutput=False)
    pf_d = nc.declare_dram_parameter("pf32", [P, NPF], F32, isOutput=False)
    out_d = nc.declare_dram_parameter("out", [B_LOC, DOUT], F32, isOutput=True)
    if DEBUG:
        dbgG_d = nc.declare_dram_parameter("dbg_G", [P, EXT], F32, isOutput=True)
        dbgM1_d = nc.declare_dram_parameter("dbg_M1", [P, P], F32, isOutput=True)

    with ExitStack() as ctx:
        tc = ctx.enter_context(tile.TileContext(nc))
        cst = ctx.enter_context(tc.tile_pool(name="cst", bufs=1))
        hex_ = ctx.enter_context(tc.tile_pool(name="hex", bufs=3))
        etm = ctx.enter_context(tc.tile_pool(name="etm", bufs=3))
        qtm = ctx.enter_context(tc.tile_pool(name="qtm", bufs=2))
        sqp = ctx.enter_context(tc.tile_pool(name="sqp", bufs=3))
        bigq = ctx.enter_context(tc.tile_pool(name="bigq", bufs=2))
        hcm = ctx.enter_context(tc.tile_pool(name="hcm", bufs=3))
        a1p = ctx.enter_context(tc.tile_pool(name="a1p", bufs=3))
        smal = ctx.enter_context(tc.tile_pool(name="smal", bufs=2))
        gps = ctx.enter_context(tc.tile_pool(name="gps", bufs=1, space="PSUM"))
        z1p = ctx.enter_context(tc.tile_pool(name="z1p", bufs=2, space="PSUM"))
        z2p = ctx.enter_context(tc.tile_pool(name="z2p", bufs=1, space="PSUM"))
        qps = ctx.enter_context(tc.tile_pool(name="qps", bufs=1, space="PSUM"))
        drp = ctx.enter_context(tc.tile_pool(name="drp", bufs=8, space="DRAM"))

        # ---- params (two host-packed tensors, one DMA each) -------------
        pbf = cst.tile([P, NPBF], BF16)
        nc.sync.dma_start(pbf[:], pbf_d[:])
        pf = cst.tile([P, NPF], F32)
        nc.sync.dma_start(pf[:], pf_d[:])
        W1rep = [pbf[:, C * i:C * (i + 1)] for i in range(NB)]
        W2bd = [pbf[:, NB * C + P * i:NB * C + P * (i + 1)] for i in range(NB)]
        b1rep = [pf[:, i:i + 1] for i in range(NB)]
        b2rep = [pf[:, NB + i:NB + i + 1] for i in range(NB)]
        o = 2 * NB
        Whrep = pf[:, o:o + C]
        Wfrep = pf[:, o + C:o + C + DOUT]
        o += C + DOUT
        bh_r = pf[:, o:o + 1]
        bng_r = pf[:, o + 1:o + 2]
        bnb_r = pf[:, o + 2:o + 3]
        bnm_r = pf[:, o + 3:o + 4]
        bnv_r = pf[:, o + 4:o + 5]
        bf_s = pf[:, o + 5:o + 6]

        # ---- constants --------------------------------------------------
        ident = cst.tile([P, P], BF16)
        make_identity(nc, ident[:])
        ones_bf = cst.tile([P, 1], BF16)
        nc.vector.memset(ones_bf[:], 1.0)
        id11 = cst.tile([1, 1], F32)
        nc.vector.memset(id11[:], 1.0)
        headmask = cst.tile([P, P], BF16)
        nc.vector.memset(headmask[:], 1.0)
        hm_v = headmask[:].rearrange("p (g i) -> p g i", i=DH)
        nc.gpsimd.affine_select(
            out=hm_v, in_=hm_v, pattern=[[-DH, P // DH], [0, DH]],
            compare_op=mybir.AluOpType.is_ge, fill=0.0,
            base=0, channel_multiplier=1)
        nc.gpsimd.affine_select(
            out=hm_v, in_=hm_v, pattern=[[DH, P // DH], [0, DH]],
            compare_op=mybir.AluOpType.is_ge, fill=0.0,
            base=DH - 1, channel_multiplier=-1)

        # BN eval folding: y_bn = y_raw * svecL + tvec, where y_raw is the
        # UNSCALED pooled-sum matmul output (missing bias and the 1/L mean).
        eps_t = cst.tile([P, 1], F32)
        nc.vector.memset(eps_t[:], BN_EPS)
        sq_t = cst.tile([P, 1], F32)
        nc.scalar.activation(sq_t[:], bnv_r, AF.Sqrt, bias=eps_t[:])
        rs_t = cst.tile([P, 1], F32)
        nc.vector.reciprocal(rs_t[:], sq_t[:])
        svec = cst.tile([P, 1], F32)
        nc.vector.tensor_tensor(svec[:], rs_t[:], bng_r, op=MUL)
        svecL = cst.tile([P, 1], F32)
        nc.vector.tensor_scalar_mul(svecL[:], svec[:], 1.0 / L)
        t0 = cst.tile([P, 1], F32)
        nc.vector.tensor_tensor(t0[:], bh_r, bnm_r,
                                op=mybir.AluOpType.subtract)
        t1 = cst.tile([P, 1], F32)
        nc.vector.tensor_tensor(t1[:], t0[:], svec[:], op=MUL)
        tvec = cst.tile([P, 1], F32)
        nc.vector.tensor_tensor(tvec[:], t1[:], bnb_r, op=ADD)

        # (repetition loop for benchmarking only; reps=1 in production)
        for _rep in range(reps):
            pooled_parts = cst.tile([P, NW], F32, tag=f"pool_{_rep}")

            def emit_exp(blk, s, he):
                """E = exp(h) for one token-major slice (Act engine)."""
                ext = EXT if blk == 0 else 128
                et = etm.tile([P, NCH * ext], BF16, tag="etm")
                nc.scalar.activation(et[:], he[:], AF.Exp)
                return et

            def emit_rest(blk, s, he, et, G_ps, q_cm):
                """Everything after exp for one slice: gram/ksum (PE),
                softmax denominator (DVE), q multiply (GPSIMD), q transpose
                (PE) + copies (DVE)."""
                ext = EXT if blk == 0 else 128
                # gram: G[d, e] += E_chunk.T @ h_chunk; block 0's ones
                # column also accumulates ksum into G[:, 128].
                for c in range(NCH):
                    nc.tensor.matmul(
                        G_ps[:, 0:ext],
                        et[:, ext * c:ext * c + 128],
                        he[:, ext * c:ext * c + ext],
                        start=(s == 0 and c == 0),
                        stop=(s == NSL - 1 and c == NCH - 1),
                    )
                if blk > 0:
                    # ksum row, folded by 256: [1,256] += ones.T @ E
                    for c in range(NCH // 2):
                        nc.tensor.matmul(
                            G_ps[0:1, 128:384],
                            ones_bf[:],
                            et[:, 256 * c:256 * (c + 1)],
                            start=(s == 0 and c == 0),
                            stop=(s == NSL - 1 and c == NCH // 2 - 1),
                        )
                # q-softmax denominator: segmented sum over d (free dim)
                if ext == 128:
                    et_dv = et[:].rearrange("p (k d) -> p k d", d=DH)
                else:
                    et_v = et[:].rearrange("p (c e) -> p c e", e=ext)
                    et_dv = et_v[:, :, 0:128].rearrange(
                        "p c (g d) -> p c g d", d=DH)
                sq = sqp.tile([P, NCH * 16], F32, tag="sq")
                sqv = (sq[:].rearrange("p k -> p k").unsqueeze(-1)
                       if ext == 128 else
                       sq[:].rearrange("p (c g) -> p c g", g=16))
                nc.vector.reduce_sum(sqv, et_dv, axis=mybir.AxisListType.X)
                rq = sqp.tile([P, NCH * 16], F32, tag="rq")
                nc.vector.reciprocal(rq[:], sq[:])
                # q = E * (1/sq) broadcast over d  (GPSIMD)
                qt = qtm.tile([P, SLC], BF16, tag="qtm")
                if ext == 128:
                    nc.gpsimd.tensor_tensor(
                        qt[:].rearrange("p (k d) -> p k d", d=DH),
                        et_dv,
                        rq[:].unsqueeze(-1).broadcast_to([P, NCH * 16, DH]),
                        op=MUL,
                    )
                else:
                    nc.gpsimd.tensor_tensor(
                        qt[:].rearrange("p (c g d) -> p c g d", g=16, d=DH),
                        et_dv,
                        rq[:].rearrange("p (c g) -> p c g", g=16)
                            .unsqueeze(-1).broadcast_to([P, NCH, 16, DH]),
                        op=MUL,
                    )
                # transpose q to channel-major via PE
                for g in range(NCH // 8):
                    qp = qps.tile([P, 1024], BF16, tag="qp")
                    for k in range(8):
                        c = 8 * g + k
                        nc.tensor.transpose(
                            qp[:, 128 * k:128 * (k + 1)],
                            qt[:, 128 * c:128 * (c + 1)],
                            ident[:],
                        )
                    nc.vector.tensor_copy(
                        q_cm[:, SLC * s + 1024 * g: SLC * s + 1024 * (g + 1)],
                        qp[:],
                    )

            # ---- block 0 phase 1 (standalone, from host-packed x) ---------
            G_ps = gps.tile([P, 384], F32, tag="G")
            q_cm = bigq.tile([P, L], BF16, tag="qcm")
            for s in range(NSL):
                he = hex_.tile([P, NCH * EXT], BF16, tag="hex")
                nc.sync.dma_start(he[:], x_d[s])
                et = etm.tile([P, NCH * EXT], BF16, tag="etm")
                nc.scalar.activation(et[:], he[:], AF.Exp)
                emit_rest(0, s, he, et, G_ps, q_cm)

            for blk in range(NB):
                # ============================ M1 build ======================
                # M1 = diag(1/ksum) @ G_m @ W1, written block-diagonally
                # into [128, 128]. ksum: block 0 from G_ps[:, 128] (ones
                # column); blocks 1-2 from the folded ksum row, transposed.
                ksC = smal.tile([P, 1], F32, tag="ksC")
                if blk == 0:
                    nc.vector.reciprocal(ksC[:], G_ps[:, 128:EXT])
                else:
                    ksr_sb = smal.tile([1, P], F32, tag="ksr_sb")
                    nc.vector.reduce_sum(
                        ksr_sb[:],
                        G_ps[0:1, 128:384].rearrange("p (a k) -> p k a", a=2),
                        axis=mybir.AxisListType.X,
                    )
                    kT_ps = gps.tile([P, C], F32, tag="G")
                    nc.tensor.transpose(kT_ps[:, 0:1], ksr_sb[:], id11[:])
                    nc.vector.reciprocal(ksC[:], kT_ps[:, 0:1])
                G_sb = smal.tile([P, P], BF16, tag="Gsb")
                nc.vector.tensor_tensor(G_sb[:], G_ps[:, 0:128], headmask[:], op=MUL)
                # per-batch 32x32 diag-block transpose on the DVE stream
                # transposer (the off-diag blocks transpose too; unread)
                Gt = smal.tile([P, P], BF16, tag="gt2sb")
                nc.vector.transpose(Gt[:], G_sb[:])
                M1u_ps = gps.tile([P, P], F32, tag="G")
                for b in range(B_LOC):
                    sl = slice(C * b, C * (b + 1))
                    nc.tensor.matmul(
                        M1u_ps[sl, C * b:C * (b + 1)], Gt[sl, C * b:C * (b + 1)],
                        W1rep[blk][sl, :],
                        tile_position=(C * b, C * b),
                    )
                if DEBUG and blk == 0:
                    dG = smal.tile([P, EXT], F32, tag="dG")
                    nc.vector.tensor_copy(dG[:], G_ps[:, 0:EXT])
                    nc.sync.dma_start(dbgG_d[:], dG[:])
                M1 = smal.tile([P, P], BF16, tag="m1")
                nc.vector.memset(M1[:], 0.0)
                for b in range(B_LOC):
                    sl = slice(C * b, C * (b + 1))
                    nc.vector.tensor_scalar_mul(
                        M1[sl, C * b:C * (b + 1)],
                        M1u_ps[sl, C * b:C * (b + 1)], ksC[sl, :])
                if DEBUG and blk == 0:
                    dM = smal.tile([P, P], F32, tag="dM")
                    nc.vector.tensor_copy(dM[:], M1[:])
                    nc.sync.dma_start(dbgM1_d[:], dM[:])

                # ============================ phase 2 (channel-major) =======
                # Software-pipelined two ways: z1/gelu1 run two windows ahead
                # of z2/gelu2, AND the NEXT block's phase 1 is interleaved
                # into this block's window loop (the engines execute their
                # queues in program order, so without this the Act engine
                # would run all gelus before any next-block exp). Exps are
                # batched (EXP_AT) to amortize the exp<->gelu activation
                # table swaps; the per-slice gram/q work (REST_AT) is spread
                # one slice per window to fit the PE queue's slack.
                last = blk == NB - 1
                EXP_AT = {7: (0, 1, 2), 13: (3, 4, 5), 15: (6,)}
                REST_AT = {9: (0,), 11: (1,), 13: (2,), 15: (3,)}
                new_h_dr = []
                he_n = [None] * NSL
                et_n = [None] * NSL
                a1t = [None] * 3
                hn = None
                if not last:
                    G_next = gps.tile([P, 384], F32, tag="G")
                    q_next = bigq.tile([P, L], BF16, tag="qcm")

                def emit_z1(v):
                    z1 = z1p.tile([P, WIN], F32, tag="z1")
                    for j in range(2):
                        nc.tensor.matmul(
                            z1[:, 512 * j:512 * (j + 1)], M1[:],
                            q_cm[:, WIN * v + 512 * j:WIN * v + 512 * (j + 1)])
                    a1 = a1p.tile([P, WIN], BF16, tag="a1")
                    nc.scalar.activation(a1[:], z1[:], AF.Gelu, bias=b1rep[blk])
                    a1t[v % 3] = a1

                for v in range(2):
                    emit_z1(v)
                for w in range(NW):
                    z2 = z2p.tile([P, WIN], F32, tag="z2")
                    for j in range(2):
                        nc.tensor.matmul(
                            z2[:, 512 * j:512 * (j + 1)], W2bd[blk],
                            a1t[w % 3][:, 512 * j:512 * (j + 1)],
                        )
                    if w % 2 == 0:
                        hn = hcm.tile([P, SLC], BF16, tag="hcm")
                    dst = hn[:, WIN * (w % 2):WIN * (w % 2 + 1)]
                    if last:
                        nc.scalar.activation(
                            dst, z2[:], AF.Gelu, bias=b2rep[blk],
                            accum_out=pooled_parts[:, w:w + 1],
                        )
                    else:
                        nc.scalar.activation(
                            dst, z2[:], AF.Gelu, bias=b2rep[blk])
                    if w % 2 == 1 and not last:
                        s = w // 2
                        hdr = drp.tile([P, SLC], BF16, tag="hdr")
                        nc.sync.dma_start(hdr[:], hn[:])
                        new_h_dr.append(hdr)
                        he = hex_.tile([P, NCH * 128], BF16, tag="hex")
                        nc.sync.dma_start_transpose(
                            out=he[:].rearrange("p (c l) -> p c l", l=128),
                            in_=hdr[:],
                        )
                        he_n[s] = he
                    if w + 2 < NW:
                        emit_z1(w + 2)
                    if not last:
                        for s in EXP_AT.get(w, ()):
                            et_n[s] = emit_exp(blk + 1, s, he_n[s])
                        for s in REST_AT.get(w, ()):
                            emit_rest(blk + 1, s, he_n[s], et_n[s],
                                      G_next, q_next)
                if not last:
                    et_n[7] = emit_exp(blk + 1, 7, he_n[7])
                    for s in (4, 5, 6, 7):
                        emit_rest(blk + 1, s, he_n[s], et_n[s],
                                  G_next, q_next)
                    G_ps, q_cm = G_next, q_next

            # ============================ head ==============================
            psum_ = smal.tile([P, 1], F32, tag="poolsum")
            nc.vector.reduce_sum(psum_[:], pooled_parts[:],
                                 axis=mybir.AxisListType.X)
            y_ps = gps.tile([P, 32], F32, tag="G")
            for b in range(B_LOC):
                sl = slice(C * b, C * (b + 1))
                nc.tensor.matmul(
                    y_ps[sl, 0:1], Whrep[sl, :], psum_[sl, :],
                    tile_position=(C * b, C * b),
                )
            ybn = smal.tile([P, 1], F32, tag="ybn")
            nc.vector.tensor_scalar(
                ybn[:], y_ps[:, 0:1], svecL[:], tvec[:], op0=MUL, op1=ADD,
            )
            yg = smal.tile([P, 1], F32, tag="yg")
            nc.scalar.activation(yg[:], ybn[:], AF.Gelu)
            o_ps = gps.tile([P, 32], F32, tag="G")
            for b in range(B_LOC):
                nc.tensor.matmul(
                    o_ps[C * b:C * b + DOUT, 0:1],
                    Wfrep[C * b:C * (b + 1), :],
                    yg[C * b:C * (b + 1), :],
                    tile_position=(C * b, C * b),
                )
            ob = smal.tile([P, 1], F32, tag="ob")
            for b in range(B_LOC):
                sl = slice(C * b, C * b + DOUT)
                nc.vector.tensor_tensor(ob[sl, :], o_ps[sl, 0:1], bf_s[sl, :], op=ADD)
            for b in range(B_LOC):
                nc.sync.dma_start(
                    out_d[b, :], ob[C * b:C * b + DOUT, 0],
                )

    _fix_sync_waits(nc)
    return nc


def _pack_x(xc):
    """[B_LOC, C, L] f32 -> [NSL, 128, NCH*129] bf16 token-major chunked
    tiles with a ones column per 128-token chunk (gram ksum extension)."""
    import ml_dtypes
    xt = np.ascontiguousarray(xc.transpose(2, 0, 1)).reshape(L, P)
    xs = xt.reshape(NSL, NCH, 128, P).transpose(0, 2, 1, 3)  # [s, t, k, col]
    out = np.empty((NSL, 128, NCH, EXT), dtype=ml_dtypes.bfloat16)
    out[..., 0:128] = xs.astype(ml_dtypes.bfloat16)
    out[..., 128] = 1.0
    return out.reshape(NSL, 128, NCH * EXT)


def _pack_params(arrs):
    """Host-side pre-replication of all weights/biases into one bf16 and one
    f32 [128, K] tensor (see build_program for the column layout)."""
    import ml_dtypes
    reps = []
    for i in range(NB):
        reps.append(np.tile(arrs["fcW1"][i], (B_LOC, 1)))          # [128,32]
    eye4 = np.eye(B_LOC, dtype=np.float32)
    for i in range(NB):
        reps.append(np.kron(eye4, arrs["fcW2"][i]))                # [128,128]
    pbf = np.concatenate(reps, axis=1).astype(ml_dtypes.bfloat16)

    cols = []
    for i in range(NB):
        cols.append(np.tile(arrs["fcb1"][i], B_LOC)[:, None])
    for i in range(NB):
        cols.append(np.tile(arrs["fcb2"][i], B_LOC)[:, None])
    cols.append(np.tile(arrs["Wh"], (B_LOC, 1)))                   # [128,32]
    cols.append(np.tile(arrs["Wf"], (B_LOC, 1)))                   # [128,10]
    for k in ("bh", "bn_gamma", "bn_beta", "bn_mean", "bn_var"):
        cols.append(np.tile(arrs[k], B_LOC)[:, None])
    bf_col = np.zeros((P, 1), np.float32)
    for b in range(B_LOC):
        bf_col[C * b:C * b + DOUT, 0] = arrs["bf"]
    cols.append(bf_col)
    pf = np.concatenate(cols, axis=1).astype(np.float32)
    assert pf.shape[1] == NPF, pf.shape
    return pbf, pf


def prep_in_maps(arrs, n_cores=8):
    x = arrs["x"]
    bl = x.shape[0] // n_cores
    pbf, pf = _pack_params(arrs)
    return [
        {"x": _pack_x(x[bl * i: bl * (i + 1)]), "pbf": pbf, "pf32": pf}
        for i in range(n_cores)
    ]


_NC_CACHE = [None]


def kernel(**inputs) -> np.ndarray:
    arrs = {k: np.asarray(v, dtype=np.float32) for k, v in inputs.items()}
    n_cores = 8

    if _NC_CACHE[0] is None:
        _NC_CACHE[0] = build_program()
    nc = _NC_CACHE[0]

    in_maps = prep_in_maps(arrs, n_cores)
    res = run_bass_kernel_spmd(nc, in_maps, list(range(n_cores))).results
    return np.concatenate([res[i]["out"] for i in range(n_cores)], axis=0)


# revision 40
# speedup vs baseline: 1.0075x; 1.0054x over previous
"""Trainium2 Bass kernel for nn_CLFormer (3-block linear-attention transformer).

Sharding: pure data parallel — batch 32 split as 4 per NeuronCore across 8
cores; all parameters replicated; outputs concatenated.

Per-core layout: 4 batches x 32 channels packed onto the 128 SBUF partitions
("channel-major" [128=4bx32c, L]). The kv-gram and k-softmax denominator
contract over tokens, so a token-major copy is needed per block: block 0
reads host-packed token-major chunked bf16 tiles of x (with a baked-in ones
column per 128-token chunk so the gram matmul also accumulates ksum);
blocks 1-2 round-trip h through DRAM and use the XBAR DMA transpose (which
writes contiguous chunks only — their ksum row comes from ones-stationary
matmuls into spare columns of the G PSUM bank). Attention-out + FC1 fuse
into M1 = diag(1/ksum)@KV@W1, held block-diagonally [128,128] so one matmul
serves all 4 batches; W2 likewise. All weights/biases arrive pre-replicated
in two host-packed tensors (2 DMAs instead of ~84 small ones).
"""
import sys
import numpy as np

for _p in ("/opt/trn_rl_repo", "/root/.axon_site/_ro/trn_rl_repo"):
    if _p not in sys.path:
        sys.path.append(_p)

from contextlib import ExitStack

import concourse.bass as bass
import concourse.mybir as mybir
import bass_rust
from concourse import tile
from concourse.masks import make_identity
from concourse.bass_utils import run_bass_kernel_spmd

F32 = mybir.dt.float32
BF16 = mybir.dt.bfloat16
AF = mybir.ActivationFunctionType
MUL = mybir.AluOpType.mult
ADD = mybir.AluOpType.add

P = 128
B_LOC = 4            # batches per core
C = 32               # channels
L = 16384            # sequence length
NB = 3               # transformer blocks
DOUT = 10
HEADS = 4
DH = 8
BN_EPS = 1e-5
DEBUG = False

SLC = 2048           # slice width (tokens per pipeline slice)
NSL = L // SLC       # 8 slices
NCH = SLC // 128     # 16 chunks per slice
EXT = 129            # chunk width incl. the ones column (block 0)
WIN = 1024           # phase-2 z2/gelu2 window width
NW = L // WIN        # 16 windows

NPBF = NB * C + NB * P        # packed bf16 param cols: W1 x3, W2bd x3
NPF = 2 * NB + C + DOUT + 6   # packed f32 param cols


# ---------------------------------------------------------------- waitfix --
_WF_SKIP = {"InstEventSemaphore"}
_wf_ctr = [0]


def _fix_sync_waits(nc):
    """Hoist excess sync waits onto InstEventSemaphore (this walrus build
    accepts only 1 wait per instruction). The event-sem executes on the same
    engine stream immediately before, preserving semantics."""
    for fn in nc.m.functions:
        new_blocks = []
        for blk in fn.blocks:
            out = []
            for ins in blk.instructions:
                tname = type(ins).__name__
                si = ins.sync_info
                if si is None or tname in _WF_SKIP:
                    out.append(ins)
                    continue
                waits = list(si.on_wait)
                if len(waits) <= 1:
                    out.append(ins)
                    continue
                keep = waits[-1:]
                excess = waits[:-1]
                for i in range(0, len(excess), 2):
                    chunk = excess[i:i + 2]
                    _wf_ctr[0] += 1
                    ev = mybir.InstEventSemaphore(
                        name=f"wfix{_wf_ctr[0]}", ins=[], outs=[])
                    ev.engine = ins.engine
                    ev.sync_info = mybir.SyncInfo(on_wait=chunk, on_update=[])
                    out.append(ev)
                ins.sync_info = mybir.SyncInfo(
                    on_wait=keep, on_update=list(si.on_update))
                out.append(ins)
            nb = bass_rust.BasicBlock(name=blk.name, instructions=out)
            new_blocks.append(nb)
        fn.blocks = new_blocks


# ---------------------------------------------------------------- program --
def build_program(reps=1):
    nc = bass.Bass()

    x_d = nc.declare_dram_parameter("x", [NSL, P, NCH * EXT], BF16, isOutput=False)
    pbf_d = nc.declare_dram_parameter("pbf", [P, NPBF], BF16, isOutput=False)
    pf_d = nc.declare_dram_parameter("pf32", [P, NPF], F32, isOutput=False)
    out_d = nc.declare_dram_parameter("out", [B_LOC, DOUT], F32, isOutput=True)
    if DEBUG:
        dbgG_d = nc.declare_dram_parameter("dbg_G", [P, EXT], F32, isOutput=True)
        dbgM1_d = nc.declare_dram_parameter("dbg_M1", [P, P], F32, isOutput=True)

    with ExitStack() as ctx:
        tc = ctx.enter_context(tile.TileContext(nc))
        cst = ctx.enter_context(tc.tile_pool(name="cst", bufs=1))
        hex_ = ctx.enter_context(tc.tile_pool(name="hex", bufs=3))
        etm = ctx.enter_context(tc.tile_pool(name="etm", bufs=3))
        qtm = ctx.enter_context(tc.tile_pool(name="qtm", bufs=2))
        sqp = ctx.enter_context(tc.tile_pool(name="sqp", bufs=3))
        bigq = ctx.enter_context(tc.tile_pool(name="bigq", bufs=2))
        hcm = ctx.enter_context(tc.tile_pool(name="hcm", bufs=3))
        a1p = ctx.enter_context(tc.tile_pool(name="a1p", bufs=3))
        smal = ctx.enter_context(tc.tile_pool(name="smal", bufs=2))
        gps = ctx.enter_context(tc.tile_pool(name="gps", bufs=1, space="PSUM"))
        z1p = ctx.enter_context(tc.tile_pool(name="z1p", bufs=2, space="PSUM"))
        z2p = ctx.enter_context(tc.tile_pool(name="z2p", bufs=1, space="PSUM"))
        qps = ctx.enter_context(tc.tile_pool(name="qps", bufs=1, space="PSUM"))
        drp = ctx.enter_context(tc.tile_pool(name="drp", bufs=8, space="DRAM"))

        # ---- params (two host-packed tensors, one DMA each) -------------
        pbf = cst.tile([P, NPBF], BF16)
        nc.sync.dma_start(pbf[:], pbf_d[:])
        pf = cst.tile([P, NPF], F32)
        nc.sync.dma_start(pf[:], pf_d[:])
        W1rep = [pbf[:, C * i:C * (i + 1)] for i in range(NB)]
        W2bd = [pbf[:, NB * C + P * i:NB * C + P * (i + 1)] for i in range(NB)]
        b1rep = [pf[:, i:i + 1] for i in range(NB)]
        b2rep = [pf[:, NB + i:NB + i + 1] for i in range(NB)]
        o = 2 * NB
        Whrep = pf[:, o:o + C]
        Wfrep = pf[:, o + C:o + C + DOUT]
        o += C + DOUT
        bh_r = pf[:, o:o + 1]
        bng_r = pf[:, o + 1:o + 2]
        bnb_r = pf[:, o + 2:o + 3]
        bnm_r = pf[:, o + 3:o + 4]
        bnv_r = pf[:, o + 4:o + 5]
        bf_s = pf[:, o + 5:o + 6]

        # ---- constants --------------------------------------------------
        ident = cst.tile([P, P], BF16)
        make_identity(nc, ident[:])
        ones_bf = cst.tile([P, 1], BF16)
        nc.vector.memset(ones_bf[:], 1.0)
        id11 = cst.tile([1, 1], F32)
        nc.vector.memset(id11[:], 1.0)
        headmask = cst.tile([P, P], BF16)
        nc.vector.memset(headmask[:], 1.0)
        hm_v = headmask[:].rearrange("p (g i) -> p g i", i=DH)
        nc.gpsimd.affine_select(
            out=hm_v, in_=hm_v, pattern=[[-DH, P // DH], [0, DH]],
            compare_op=mybir.AluOpType.is_ge, fill=0.0,
            base=0, channel_multiplier=1)
        nc.gpsimd.affine_select(
            out=hm_v, in_=hm_v, pattern=[[DH, P // DH], [0, DH]],
            compare_op=mybir.AluOpType.is_ge, fill=0.0,
            base=DH - 1, channel_multiplier=-1)

        # BN eval folding: y_bn = y_raw * svecL + tvec, where y_raw is the
        # UNSCALED pooled-sum matmul output (missing bias and the 1/L mean).
        eps_t = cst.tile([P, 1], F32)
        nc.vector.memset(eps_t[:], BN_EPS)
        sq_t = cst.tile([P, 1], F32)
        nc.scalar.activation(sq_t[:], bnv_r, AF.Sqrt, bias=eps_t[:])
        rs_t = cst.tile([P, 1], F32)
        nc.vector.reciprocal(rs_t[:], sq_t[:])
        svec = cst.tile([P, 1], F32)
        nc.vector.tensor_tensor(svec[:], rs_t[:], bng_r, op=MUL)
        svecL = cst.tile([P, 1], F32)
        nc.vector.tensor_scalar_mul(svecL[:], svec[:], 1.0 / L)
        t0 = cst.tile([P, 1], F32)
        nc.vector.tensor_tensor(t0[:], bh_r, bnm_r,
                                op=mybir.AluOpType.subtract)
        t1 = cst.tile([P, 1], F32)
        nc.vector.tensor_tensor(t1[:], t0[:], svec[:], op=MUL)
        tvec = cst.tile([P, 1], F32)
        nc.vector.tensor_tensor(tvec[:], t1[:], bnb_r, op=ADD)

        # (repetition loop for benchmarking only; reps=1 in production)
        for _rep in range(reps):
            pooled_parts = cst.tile([P, NW], F32, tag=f"pool_{_rep}")

            def emit_exp(blk, s, he):
                """E = exp(h) for one token-major slice (Act engine)."""
                ext = EXT if blk == 0 else 128
                et = etm.tile([P, NCH * ext], BF16, tag="etm")
                nc.scalar.activation(et[:], he[:], AF.Exp)
                return et

            def emit_rest(blk, s, he, et, G_ps, q_cm):
                """Everything after exp for one slice: gram/ksum (PE),
                softmax denominator (DVE), q multiply (GPSIMD), q transpose
                (PE) + copies (DVE)."""
                ext = EXT if blk == 0 else 128
                # gram: G[d, e] += E_chunk.T @ h_chunk; block 0's ones
                # column also accumulates ksum into G[:, 128].
                for c in range(NCH):
                    nc.tensor.matmul(
                        G_ps[:, 0:ext],
                        et[:, ext * c:ext * c + 128],
                        he[:, ext * c:ext * c + ext],
                        start=(s == 0 and c == 0),
                        stop=(s == NSL - 1 and c == NCH - 1),
                    )
                if blk > 0:
                    # ksum row, folded by 256: [1,256] += ones.T @ E
                    for c in range(NCH // 2):
                        nc.tensor.matmul(
                            G_ps[0:1, 128:384],
                            ones_bf[:],
                            et[:, 256 * c:256 * (c + 1)],
                            start=(s == 0 and c == 0),
                            stop=(s == NSL - 1 and c == NCH // 2 - 1),
                        )
                # q-softmax denominator: segmented sum over d (free dim)
                et_v = et[:].rearrange("p (c e) -> p c e", e=ext)
                et_dv = et_v[:, :, 0:128].rearrange(
                    "p c (g d) -> p c g d", d=DH)
                sq = sqp.tile([P, NCH * 16], F32, tag="sq")
                nc.vector.reduce_sum(
                    sq[:].rearrange("p (c g) -> p c g", g=16),
                    et_dv,
                    axis=mybir.AxisListType.X,
                )
                rq = sqp.tile([P, NCH * 16], F32, tag="rq")
                nc.vector.reciprocal(rq[:], sq[:])
                # q = E * (1/sq) broadcast over d  (GPSIMD)
                qt = qtm.tile([P, SLC], BF16, tag="qtm")
                nc.gpsimd.tensor_tensor(
                    qt[:].rearrange("p (c g d) -> p c g d", g=16, d=DH),
                    et_dv,
                    rq[:].rearrange("p (c g) -> p c g", g=16)
                        .unsqueeze(-1).broadcast_to([P, NCH, 16, DH]),
                    op=MUL,
                )
                # transpose q to channel-major via PE
                for g in range(NCH // 8):
                    qp = qps.tile([P, 1024], BF16, tag="qp")
                    for k in range(8):
                        c = 8 * g + k
                        nc.tensor.transpose(
                            qp[:, 128 * k:128 * (k + 1)],
                            qt[:, 128 * c:128 * (c + 1)],
                            ident[:],
                        )
                    nc.vector.tensor_copy(
                        q_cm[:, SLC * s + 1024 * g: SLC * s + 1024 * (g + 1)],
                        qp[:],
                    )

            # ---- block 0 phase 1 (standalone, from host-packed x) ---------
            G_ps = gps.tile([P, 384], F32, tag="G")
            q_cm = bigq.tile([P, L], BF16, tag="qcm")
            for s in range(NSL):
                he = hex_.tile([P, NCH * EXT], BF16, tag="hex")
                nc.sync.dma_start(he[:], x_d[s])
                et = etm.tile([P, NCH * EXT], BF16, tag="etm")
                nc.scalar.activation(et[:], he[:], AF.Exp)
                emit_rest(0, s, he, et, G_ps, q_cm)

            for blk in range(NB):
                # ============================ M1 build ======================
                # M1 = diag(1/ksum) @ G_m @ W1, written block-diagonally
                # into [128, 128]. ksum: block 0 from G_ps[:, 128] (ones
                # column); blocks 1-2 from the folded ksum row, transposed.
                ksC = smal.tile([P, 1], F32, tag="ksC")
                if blk == 0:
                    nc.vector.reciprocal(ksC[:], G_ps[:, 128:EXT])
                else:
                    ksr_sb = smal.tile([1, P], F32, tag="ksr_sb")
                    nc.vector.reduce_sum(
                        ksr_sb[:],
                        G_ps[0:1, 128:384].rearrange("p (a k) -> p k a", a=2),
                        axis=mybir.AxisListType.X,
                    )
                    kT_ps = gps.tile([P, C], F32, tag="G")
                    nc.tensor.transpose(kT_ps[:, 0:1], ksr_sb[:], id11[:])
                    nc.vector.reciprocal(ksC[:], kT_ps[:, 0:1])
                G_sb = smal.tile([P, P], BF16, tag="Gsb")
                nc.vector.tensor_tensor(G_sb[:], G_ps[:, 0:128], headmask[:], op=MUL)
                # per-batch 32x32 diag-block transpose on the DVE stream
                # transposer (the off-diag blocks transpose too; unread)
                Gt = smal.tile([P, P], BF16, tag="gt2sb")
                nc.vector.transpose(Gt[:], G_sb[:])
                M1u_ps = gps.tile([P, P], F32, tag="G")
                for b in range(B_LOC):
                    sl = slice(C * b, C * (b + 1))
                    nc.tensor.matmul(
                        M1u_ps[sl, C * b:C * (b + 1)], Gt[sl, C * b:C * (b + 1)],
                        W1rep[blk][sl, :],
                        tile_position=(C * b, C * b),
                    )
                if DEBUG and blk == 0:
                    dG = smal.tile([P, EXT], F32, tag="dG")
                    nc.vector.tensor_copy(dG[:], G_ps[:, 0:EXT])
                    nc.sync.dma_start(dbgG_d[:], dG[:])
                M1 = smal.tile([P, P], BF16, tag="m1")
                nc.vector.memset(M1[:], 0.0)
                for b in range(B_LOC):
                    sl = slice(C * b, C * (b + 1))
                    nc.vector.tensor_scalar_mul(
                        M1[sl, C * b:C * (b + 1)],
                        M1u_ps[sl, C * b:C * (b + 1)], ksC[sl, :])
                if DEBUG and blk == 0:
                    dM = smal.tile([P, P], F32, tag="dM")
                    nc.vector.tensor_copy(dM[:], M1[:])
                    nc.sync.dma_start(dbgM1_d[:], dM[:])

                # ============================ phase 2 (channel-major) =======
                # Software-pipelined two ways: z1/gelu1 run two windows ahead
                # of z2/gelu2, AND the NEXT block's phase 1 is interleaved
                # into this block's window loop (the engines execute their
                # queues in program order, so without this the Act engine
                # would run all gelus before any next-block exp). Exps are
                # batched (EXP_AT) to amortize the exp<->gelu activation
                # table swaps; the per-slice gram/q work (REST_AT) is spread
                # one slice per window to fit the PE queue's slack.
                last = blk == NB - 1
                EXP_AT = {7: (0, 1, 2), 13: (3, 4, 5), 15: (6,)}
                REST_AT = {9: (0,), 11: (1,), 13: (2,), 15: (3,)}
                new_h_dr = []
                he_n = [None] * NSL
                et_n = [None] * NSL
                a1t = [None] * 3
                hn = None
                if not last:
                    G_next = gps.tile([P, 384], F32, tag="G")
                    q_next = bigq.tile([P, L], BF16, tag="qcm")

                def emit_z1(v):
                    z1 = z1p.tile([P, WIN], F32, tag="z1")
                    for j in range(2):
                        nc.tensor.matmul(
                            z1[:, 512 * j:512 * (j + 1)], M1[:],
                            q_cm[:, WIN * v + 512 * j:WIN * v + 512 * (j + 1)])
                    a1 = a1p.tile([P, WIN], BF16, tag="a1")
                    nc.scalar.activation(a1[:], z1[:], AF.Gelu, bias=b1rep[blk])
                    a1t[v % 3] = a1

                for v in range(2):
                    emit_z1(v)
                for w in range(NW):
                    z2 = z2p.tile([P, WIN], F32, tag="z2")
                    for j in range(2):
                        nc.tensor.matmul(
                            z2[:, 512 * j:512 * (j + 1)], W2bd[blk],
                            a1t[w % 3][:, 512 * j:512 * (j + 1)],
                        )
                    if w % 2 == 0:
                        hn = hcm.tile([P, SLC], BF16, tag="hcm")
                    dst = hn[:, WIN * (w % 2):WIN * (w % 2 + 1)]
                    if last:
                        nc.scalar.activation(
                            dst, z2[:], AF.Gelu, bias=b2rep[blk],
                            accum_out=pooled_parts[:, w:w + 1],
                        )
                    else:
                        nc.scalar.activation(
                            dst, z2[:], AF.Gelu, bias=b2rep[blk])
                    if w % 2 == 1 and not last:
                        s = w // 2
                        hdr = drp.tile([P, SLC], BF16, tag="hdr")
                        nc.sync.dma_start(hdr[:], hn[:])
                        new_h_dr.append(hdr)
                        he = hex_.tile([P, NCH * 128], BF16, tag="hex")
                        nc.sync.dma_start_transpose(
                            out=he[:].rearrange("p (c l) -> p c l", l=128),
                            in_=hdr[:],
                        )
                        he_n[s] = he
                    if w + 2 < NW:
                        emit_z1(w + 2)
                    if not last:
                        for s in EXP_AT.get(w, ()):
                            et_n[s] = emit_exp(blk + 1, s, he_n[s])
                        for s in REST_AT.get(w, ()):
                            emit_rest(blk + 1, s, he_n[s], et_n[s],
                                      G_next, q_next)
                if not last:
                    et_n[7] = emit_exp(blk + 1, 7, he_n[7])
                    for s in (4, 5, 6, 7):
                        emit_rest(blk + 1, s, he_n[s], et_n[s],
                                  G_next, q_next)
                    G_ps, q_cm = G_next, q_next

            # ============================ head ==============================
            psum_ = smal.tile([P, 1], F32, tag="poolsum")
            nc.vector.reduce_sum(psum_[:], pooled_parts[:],
                                 axis=mybir.AxisListType.X)
            y_ps = gps.tile([P, 32], F32, tag="G")
            for b in range(B_LOC):
                sl = slice(C * b, C * (b + 1))
                nc.tensor.matmul(
                    y_ps[sl, 0:1], Whrep[sl, :], psum_[sl, :],
                    tile_position=(C * b, C * b),
                )
            ybn = smal.tile([P, 1], F32, tag="ybn")
            nc.vector.tensor_scalar(
                ybn[:], y_ps[:, 0:1], svecL[:], tvec[:], op0=MUL, op1=ADD,
            )
            yg = smal.tile([P, 1], F32, tag="yg")
            nc.scalar.activation(yg[:], ybn[:], AF.Gelu)
            o_ps = gps.tile([P, 32], F32, tag="G")
            for b in range(B_LOC):
                nc.tensor.matmul(
                    o_ps[C * b:C * b + DOUT, 0:1],
                    Wfrep[C * b:C * (b + 1), :],
                    yg[C * b:C * (b + 1), :],
                    tile_position=(C * b, C * b),
                )
            ob = smal.tile([P, 1], F32, tag="ob")
            for b in range(B_LOC):
                sl = slice(C * b, C * b + DOUT)
                nc.vector.tensor_tensor(ob[sl, :], o_ps[sl, 0:1], bf_s[sl, :], op=ADD)
            for b in range(B_LOC):
                nc.sync.dma_start(
                    out_d[b, :], ob[C * b:C * b + DOUT, 0],
                )

    _fix_sync_waits(nc)
    return nc


def _pack_x(xc):
    """[B_LOC, C, L] f32 -> [NSL, 128, NCH*129] bf16 token-major chunked
    tiles with a ones column per 128-token chunk (gram ksum extension)."""
    import ml_dtypes
    xt = np.ascontiguousarray(xc.transpose(2, 0, 1)).reshape(L, P)
    xs = xt.reshape(NSL, NCH, 128, P).transpose(0, 2, 1, 3)  # [s, t, k, col]
    out = np.empty((NSL, 128, NCH, EXT), dtype=ml_dtypes.bfloat16)
    out[..., 0:128] = xs.astype(ml_dtypes.bfloat16)
    out[..., 128] = 1.0
    return out.reshape(NSL, 128, NCH * EXT)


def _pack_params(arrs):
    """Host-side pre-replication of all weights/biases into one bf16 and one
    f32 [128, K] tensor (see build_program for the column layout)."""
    import ml_dtypes
    reps = []
    for i in range(NB):
        reps.append(np.tile(arrs["fcW1"][i], (B_LOC, 1)))          # [128,32]
    eye4 = np.eye(B_LOC, dtype=np.float32)
    for i in range(NB):
        reps.append(np.kron(eye4, arrs["fcW2"][i]))                # [128,128]
    pbf = np.concatenate(reps, axis=1).astype(ml_dtypes.bfloat16)

    cols = []
    for i in range(NB):
        cols.append(np.tile(arrs["fcb1"][i], B_LOC)[:, None])
    for i in range(NB):
        cols.append(np.tile(arrs["fcb2"][i], B_LOC)[:, None])
    cols.append(np.tile(arrs["Wh"], (B_LOC, 1)))                   # [128,32]
    cols.append(np.tile(arrs["Wf"], (B_LOC, 1)))                   # [128,10]
    for k in ("bh", "bn_gamma", "bn_beta", "bn_mean", "bn_var"):
        cols.append(np.tile(arrs[k], B_LOC)[:, None])
    bf_col = np.zeros((P, 1), np.float32)
    for b in range(B_LOC):
        bf_col[C * b:C * b + DOUT, 0] = arrs["bf"]
    cols.append(bf_col)
    pf = np.concatenate(cols, axis=1).astype(np.float32)
    assert pf.shape[1] == NPF, pf.shape
    return pbf, pf


def prep_in_maps(arrs, n_cores=8):
    x = arrs["x"]
    bl = x.shape[0] // n_cores
    pbf, pf = _pack_params(arrs)
    return [
        {"x": _pack_x(x[bl * i: bl * (i + 1)]), "pbf": pbf, "pf32": pf}
        for i in range(n_cores)
    ]


_NC_CACHE = [None]


def kernel(**inputs) -> np.ndarray:
    arrs = {k: np.asarray(v, dtype=np.float32) for k, v in inputs.items()}
    n_cores = 8

    if _NC_CACHE[0] is None:
        _NC_CACHE[0] = build_program()
    nc = _NC_CACHE[0]

    in_maps = prep_in_maps(arrs, n_cores)
    res = run_bass_kernel_spmd(nc, in_maps, list(range(n_cores))).results
    return np.concatenate([res[i]["out"] for i in range(n_cores)], axis=0)


# revision 41
# speedup vs baseline: 1.0412x; 1.0334x over previous
"""Trainium2 Bass kernel for nn_CLFormer (3-block linear-attention transformer).

Sharding: pure data parallel — batch 32 split as 4 per NeuronCore across 8
cores; all parameters replicated; outputs concatenated.

Per-core layout: 4 batches x 32 channels packed onto the 128 SBUF partitions
("channel-major" [128=4bx32c, L]). The kv-gram and k-softmax denominator
contract over tokens, so a token-major copy is needed per block: block 0
reads host-packed token-major chunked bf16 tiles of x (with a baked-in ones
column per 128-token chunk so the gram matmul also accumulates ksum);
blocks 1-2 round-trip h through DRAM and use the XBAR DMA transpose (which
writes contiguous chunks only — their ksum row comes from ones-stationary
matmuls into spare columns of the G PSUM bank). Attention-out + FC1 fuse
into M1 = diag(1/ksum)@KV@W1, held block-diagonally [128,128] so one matmul
serves all 4 batches; W2 likewise. All weights/biases arrive pre-replicated
in two host-packed tensors (2 DMAs instead of ~84 small ones).
"""
import sys
import numpy as np

for _p in ("/opt/trn_rl_repo", "/root/.axon_site/_ro/trn_rl_repo"):
    if _p not in sys.path:
        sys.path.append(_p)

from contextlib import ExitStack

import concourse.bass as bass
import concourse.mybir as mybir
import bass_rust
from concourse import tile
from concourse.masks import make_identity
from concourse.bass_utils import run_bass_kernel_spmd

F32 = mybir.dt.float32
BF16 = mybir.dt.bfloat16
AF = mybir.ActivationFunctionType
MUL = mybir.AluOpType.mult
ADD = mybir.AluOpType.add

P = 128
B_LOC = 4            # batches per core
C = 32               # channels
L = 16384            # sequence length
NB = 3               # transformer blocks
DOUT = 10
HEADS = 4
DH = 8
BN_EPS = 1e-5
DEBUG = False

SLC = 2048           # slice width (tokens per pipeline slice)
NSL = L // SLC       # 8 slices
NCH = SLC // 128     # 16 chunks per slice
EXT = 129            # chunk width incl. the ones column (block 0)
WIN = 1024           # phase-2 z2/gelu2 window width
NW = L // WIN        # 16 windows

NPBF = NB * C + NB * P        # packed bf16 param cols: W1 x3, W2bd x3
NPF = 2 * NB + C + DOUT + 6   # packed f32 param cols


# ---------------------------------------------------------------- waitfix --
_WF_SKIP = {"InstEventSemaphore"}
_wf_ctr = [0]


def _fix_sync_waits(nc):
    """Hoist excess sync waits onto InstEventSemaphore (this walrus build
    accepts only 1 wait per instruction). The event-sem executes on the same
    engine stream immediately before, preserving semantics."""
    for fn in nc.m.functions:
        new_blocks = []
        for blk in fn.blocks:
            out = []
            for ins in blk.instructions:
                tname = type(ins).__name__
                si = ins.sync_info
                if si is None or tname in _WF_SKIP:
                    out.append(ins)
                    continue
                waits = list(si.on_wait)
                if len(waits) <= 1:
                    out.append(ins)
                    continue
                keep = waits[-1:]
                excess = waits[:-1]
                for i in range(0, len(excess), 2):
                    chunk = excess[i:i + 2]
                    _wf_ctr[0] += 1
                    ev = mybir.InstEventSemaphore(
                        name=f"wfix{_wf_ctr[0]}", ins=[], outs=[])
                    ev.engine = ins.engine
                    ev.sync_info = mybir.SyncInfo(on_wait=chunk, on_update=[])
                    out.append(ev)
                ins.sync_info = mybir.SyncInfo(
                    on_wait=keep, on_update=list(si.on_update))
                out.append(ins)
            nb = bass_rust.BasicBlock(name=blk.name, instructions=out)
            new_blocks.append(nb)
        fn.blocks = new_blocks


# ---------------------------------------------------------------- program --
def build_program(reps=1):
    nc = bass.Bass()

    x_d = nc.declare_dram_parameter("x", [NSL, P, NCH * EXT], BF16, isOutput=False)
    pbf_d = nc.declare_dram_parameter("pbf", [P, NPBF], BF16, isOutput=False)
    pf_d = nc.declare_dram_parameter("pf32", [P, NPF], F32, isOutput=False)
    out_d = nc.declare_dram_parameter("out", [B_LOC, DOUT], F32, isOutput=True)
    if DEBUG:
        dbgG_d = nc.declare_dram_parameter("dbg_G", [P, EXT], F32, isOutput=True)
        dbgM1_d = nc.declare_dram_parameter("dbg_M1", [P, P], F32, isOutput=True)

    with ExitStack() as ctx:
        tc = ctx.enter_context(tile.TileContext(nc))
        cst = ctx.enter_context(tc.tile_pool(name="cst", bufs=1))
        hex_ = ctx.enter_context(tc.tile_pool(name="hex", bufs=3))
        etm = ctx.enter_context(tc.tile_pool(name="etm", bufs=3))
        qtm = ctx.enter_context(tc.tile_pool(name="qtm", bufs=2))
        sqp = ctx.enter_context(tc.tile_pool(name="sqp", bufs=3))
        bigq = ctx.enter_context(tc.tile_pool(name="bigq", bufs=2))
        hcm = ctx.enter_context(tc.tile_pool(name="hcm", bufs=3))
        a1p = ctx.enter_context(tc.tile_pool(name="a1p", bufs=3))
        smal = ctx.enter_context(tc.tile_pool(name="smal", bufs=2))
        gps = ctx.enter_context(tc.tile_pool(name="gps", bufs=1, space="PSUM"))
        z1p = ctx.enter_context(tc.tile_pool(name="z1p", bufs=1, space="PSUM"))
        z2p = ctx.enter_context(tc.tile_pool(name="z2p", bufs=2, space="PSUM"))
        qps = ctx.enter_context(tc.tile_pool(name="qps", bufs=1, space="PSUM"))
        drp = ctx.enter_context(tc.tile_pool(name="drp", bufs=8, space="DRAM"))

        # ---- params (two host-packed tensors, one DMA each) -------------
        pbf = cst.tile([P, NPBF], BF16)
        nc.sync.dma_start(pbf[:], pbf_d[:])
        pf = cst.tile([P, NPF], F32)
        nc.sync.dma_start(pf[:], pf_d[:])
        W1rep = [pbf[:, C * i:C * (i + 1)] for i in range(NB)]
        W2bd = [pbf[:, NB * C + P * i:NB * C + P * (i + 1)] for i in range(NB)]
        b1rep = [pf[:, i:i + 1] for i in range(NB)]
        b2rep = [pf[:, NB + i:NB + i + 1] for i in range(NB)]
        o = 2 * NB
        Whrep = pf[:, o:o + C]
        Wfrep = pf[:, o + C:o + C + DOUT]
        o += C + DOUT
        bh_r = pf[:, o:o + 1]
        bng_r = pf[:, o + 1:o + 2]
        bnb_r = pf[:, o + 2:o + 3]
        bnm_r = pf[:, o + 3:o + 4]
        bnv_r = pf[:, o + 4:o + 5]
        bf_s = pf[:, o + 5:o + 6]

        # ---- constants --------------------------------------------------
        ident = cst.tile([P, P], BF16)
        make_identity(nc, ident[:])
        ones_bf = cst.tile([P, 1], BF16)
        nc.vector.memset(ones_bf[:], 1.0)
        id11 = cst.tile([1, 1], F32)
        nc.vector.memset(id11[:], 1.0)
        headmask = cst.tile([P, P], BF16)
        nc.vector.memset(headmask[:], 1.0)
        hm_v = headmask[:].rearrange("p (g i) -> p g i", i=DH)
        nc.gpsimd.affine_select(
            out=hm_v, in_=hm_v, pattern=[[-DH, P // DH], [0, DH]],
            compare_op=mybir.AluOpType.is_ge, fill=0.0,
            base=0, channel_multiplier=1)
        nc.gpsimd.affine_select(
            out=hm_v, in_=hm_v, pattern=[[DH, P // DH], [0, DH]],
            compare_op=mybir.AluOpType.is_ge, fill=0.0,
            base=DH - 1, channel_multiplier=-1)

        # BN eval folding: y_bn = y_raw * svecL + tvec, where y_raw is the
        # UNSCALED pooled-sum matmul output (missing bias and the 1/L mean).
        eps_t = cst.tile([P, 1], F32)
        nc.vector.memset(eps_t[:], BN_EPS)
        sq_t = cst.tile([P, 1], F32)
        nc.scalar.activation(sq_t[:], bnv_r, AF.Sqrt, bias=eps_t[:])
        rs_t = cst.tile([P, 1], F32)
        nc.vector.reciprocal(rs_t[:], sq_t[:])
        svec = cst.tile([P, 1], F32)
        nc.vector.tensor_tensor(svec[:], rs_t[:], bng_r, op=MUL)
        svecL = cst.tile([P, 1], F32)
        nc.vector.tensor_scalar_mul(svecL[:], svec[:], 1.0 / L)
        t0 = cst.tile([P, 1], F32)
        nc.vector.tensor_tensor(t0[:], bh_r, bnm_r,
                                op=mybir.AluOpType.subtract)
        t1 = cst.tile([P, 1], F32)
        nc.vector.tensor_tensor(t1[:], t0[:], svec[:], op=MUL)
        tvec = cst.tile([P, 1], F32)
        nc.vector.tensor_tensor(tvec[:], t1[:], bnb_r, op=ADD)

        # (repetition loop for benchmarking only; reps=1 in production)
        for _rep in range(reps):
            pooled_parts = cst.tile([P, NW], F32, tag=f"pool_{_rep}")

            def emit_exp(blk, s, he):
                """E = exp(h) for one token-major slice (Act engine)."""
                ext = EXT if blk == 0 else 128
                et = etm.tile([P, NCH * ext], BF16, tag="etm")
                nc.scalar.activation(et[:], he[:], AF.Exp)
                return et

            def emit_rest(blk, s, he, et, G_ps, q_cm):
                """Everything after exp for one slice: gram/ksum (PE),
                softmax denominator (DVE), q multiply (GPSIMD), q transpose
                (PE) + copies (DVE)."""
                ext = EXT if blk == 0 else 128
                # gram: G[d, e] += E_chunk.T @ h_chunk; block 0's ones
                # column also accumulates ksum into G[:, 128].
                for c in range(NCH):
                    nc.tensor.matmul(
                        G_ps[:, 0:ext],
                        et[:, ext * c:ext * c + 128],
                        he[:, ext * c:ext * c + ext],
                        start=(s == 0 and c == 0),
                        stop=(s == NSL - 1 and c == NCH - 1),
                    )
                if blk > 0:
                    # ksum row, folded by 256: [1,256] += ones.T @ E
                    for c in range(NCH // 2):
                        nc.tensor.matmul(
                            G_ps[0:1, 128:384],
                            ones_bf[:],
                            et[:, 256 * c:256 * (c + 1)],
                            start=(s == 0 and c == 0),
                            stop=(s == NSL - 1 and c == NCH // 2 - 1),
                        )
                # q-softmax denominator: segmented sum over d (free dim)
                et_v = et[:].rearrange("p (c e) -> p c e", e=ext)
                et_dv = et_v[:, :, 0:128].rearrange(
                    "p c (g d) -> p c g d", d=DH)
                sq = sqp.tile([P, NCH * 16], F32, tag="sq")
                nc.vector.reduce_sum(
                    sq[:].rearrange("p (c g) -> p c g", g=16),
                    et_dv,
                    axis=mybir.AxisListType.X,
                )
                rq = sqp.tile([P, NCH * 16], F32, tag="rq")
                nc.vector.reciprocal(rq[:], sq[:])
                # q = E * (1/sq) broadcast over d  (GPSIMD)
                qt = qtm.tile([P, SLC], BF16, tag="qtm")
                nc.gpsimd.tensor_tensor(
                    qt[:].rearrange("p (c g d) -> p c g d", g=16, d=DH),
                    et_dv,
                    rq[:].rearrange("p (c g) -> p c g", g=16)
                        .unsqueeze(-1).broadcast_to([P, NCH, 16, DH]),
                    op=MUL,
                )
                # transpose q to channel-major via PE
                for g in range(NCH // 8):
                    qp = qps.tile([P, 1024], BF16, tag="qp")
                    for k in range(8):
                        c = 8 * g + k
                        nc.tensor.transpose(
                            qp[:, 128 * k:128 * (k + 1)],
                            qt[:, 128 * c:128 * (c + 1)],
                            ident[:],
                        )
                    nc.vector.tensor_copy(
                        q_cm[:, SLC * s + 1024 * g: SLC * s + 1024 * (g + 1)],
                        qp[:],
                    )

            # ---- block 0 phase 1 (standalone, from host-packed x) ---------
            G_ps = gps.tile([P, 384], F32, tag="G")
            q_cm = bigq.tile([P, L], BF16, tag="qcm")
            for s in range(NSL):
                he = hex_.tile([P, NCH * EXT], BF16, tag="hex")
                nc.sync.dma_start(he[:], x_d[s])
                et = etm.tile([P, NCH * EXT], BF16, tag="etm")
                nc.scalar.activation(et[:], he[:], AF.Exp)
                emit_rest(0, s, he, et, G_ps, q_cm)

            for blk in range(NB):
                # ============================ M1 build ======================
                # M1 = diag(1/ksum) @ G_m @ W1, written block-diagonally
                # into [128, 128]. ksum: block 0 from G_ps[:, 128] (ones
                # column); blocks 1-2 from the folded ksum row, transposed.
                ksC = smal.tile([P, 1], F32, tag="ksC")
                if blk == 0:
                    nc.vector.reciprocal(ksC[:], G_ps[:, 128:EXT])
                else:
                    ksr_sb = smal.tile([1, P], F32, tag="ksr_sb")
                    nc.vector.reduce_sum(
                        ksr_sb[:],
                        G_ps[0:1, 128:384].rearrange("p (a k) -> p k a", a=2),
                        axis=mybir.AxisListType.X,
                    )
                    kT_ps = gps.tile([P, C], F32, tag="G")
                    nc.tensor.transpose(kT_ps[:, 0:1], ksr_sb[:], id11[:])
                    nc.vector.reciprocal(ksC[:], kT_ps[:, 0:1])
                G_sb = smal.tile([P, P], BF16, tag="Gsb")
                nc.vector.tensor_tensor(G_sb[:], G_ps[:, 0:128], headmask[:], op=MUL)
                # per-batch 32x32 diag-block transpose on the DVE stream
                # transposer (the off-diag blocks transpose too; unread)
                Gt = smal.tile([P, P], BF16, tag="gt2sb")
                nc.vector.transpose(Gt[:], G_sb[:])
                M1u_ps = gps.tile([P, P], F32, tag="G")
                for b in range(B_LOC):
                    sl = slice(C * b, C * (b + 1))
                    nc.tensor.matmul(
                        M1u_ps[sl, C * b:C * (b + 1)], Gt[sl, C * b:C * (b + 1)],
                        W1rep[blk][sl, :],
                        tile_position=(C * b, C * b),
                    )
                if DEBUG and blk == 0:
                    dG = smal.tile([P, EXT], F32, tag="dG")
                    nc.vector.tensor_copy(dG[:], G_ps[:, 0:EXT])
                    nc.sync.dma_start(dbgG_d[:], dG[:])
                M1 = smal.tile([P, P], BF16, tag="m1")
                nc.vector.memset(M1[:], 0.0)
                for b in range(B_LOC):
                    sl = slice(C * b, C * (b + 1))
                    nc.vector.tensor_scalar_mul(
                        M1[sl, C * b:C * (b + 1)],
                        M1u_ps[sl, C * b:C * (b + 1)], ksC[sl, :])
                if DEBUG and blk == 0:
                    dM = smal.tile([P, P], F32, tag="dM")
                    nc.vector.tensor_copy(dM[:], M1[:])
                    nc.sync.dma_start(dbgM1_d[:], dM[:])

                # ============================ phase 2 (channel-major) =======
                # Software-pipelined two ways: z1/gelu1 run two windows ahead
                # of z2/gelu2, AND the NEXT block's phase 1 is interleaved
                # into this block's window loop (the engines execute their
                # queues in program order, so without this the Act engine
                # would run all gelus before any next-block exp). Exps are
                # batched (EXP_AT) to amortize the exp<->gelu activation
                # table swaps; the per-slice gram/q work (REST_AT) is spread
                # one slice per window to fit the PE queue's slack.
                last = blk == NB - 1
                EXP_AT = {7: (0, 1, 2), 13: (3, 4, 5), 15: (6,)}
                REST_AT = {9: (0,), 11: (1,), 13: (2,), 15: (3,)}
                new_h_dr = []
                he_n = [None] * NSL
                et_n = [None] * NSL
                a1t = [None] * 3
                hn = None
                if not last:
                    G_next = gps.tile([P, 384], F32, tag="G")
                    q_next = bigq.tile([P, L], BF16, tag="qcm")

                def emit_z1(v):
                    z1 = z1p.tile([P, WIN], F32, tag="z1")
                    for j in range(2):
                        nc.tensor.matmul(
                            z1[:, 512 * j:512 * (j + 1)], M1[:],
                            q_cm[:, WIN * v + 512 * j:WIN * v + 512 * (j + 1)])
                    a1 = a1p.tile([P, WIN], BF16, tag="a1")
                    nc.scalar.activation(a1[:], z1[:], AF.Gelu, bias=b1rep[blk])
                    a1t[v % 3] = a1

                for v in range(2):
                    emit_z1(v)
                for w in range(NW):
                    z2 = z2p.tile([P, WIN], F32, tag="z2")
                    for j in range(2):
                        nc.tensor.matmul(
                            z2[:, 512 * j:512 * (j + 1)], W2bd[blk],
                            a1t[w % 3][:, 512 * j:512 * (j + 1)],
                        )
                    if w % 2 == 0:
                        hn = hcm.tile([P, SLC], BF16, tag="hcm")
                    dst = hn[:, WIN * (w % 2):WIN * (w % 2 + 1)]
                    if last:
                        nc.scalar.activation(
                            dst, z2[:], AF.Gelu, bias=b2rep[blk],
                            accum_out=pooled_parts[:, w:w + 1],
                        )
                    else:
                        nc.scalar.activation(
                            dst, z2[:], AF.Gelu, bias=b2rep[blk])
                    if w % 2 == 1 and not last:
                        s = w // 2
                        hdr = drp.tile([P, SLC], BF16, tag="hdr")
                        nc.sync.dma_start(hdr[:], hn[:])
                        new_h_dr.append(hdr)
                        he = hex_.tile([P, NCH * 128], BF16, tag="hex")
                        nc.sync.dma_start_transpose(
                            out=he[:].rearrange("p (c l) -> p c l", l=128),
                            in_=hdr[:],
                        )
                        he_n[s] = he
                    if w + 2 < NW:
                        emit_z1(w + 2)
                    if not last:
                        for s in EXP_AT.get(w, ()):
                            et_n[s] = emit_exp(blk + 1, s, he_n[s])
                        for s in REST_AT.get(w, ()):
                            emit_rest(blk + 1, s, he_n[s], et_n[s],
                                      G_next, q_next)
                if not last:
                    et_n[7] = emit_exp(blk + 1, 7, he_n[7])
                    for s in (4, 5, 6, 7):
                        emit_rest(blk + 1, s, he_n[s], et_n[s],
                                  G_next, q_next)
                    G_ps, q_cm = G_next, q_next

            # ============================ head ==============================
            psum_ = smal.tile([P, 1], F32, tag="poolsum")
            nc.vector.reduce_sum(psum_[:], pooled_parts[:],
                                 axis=mybir.AxisListType.X)
            y_ps = gps.tile([P, 32], F32, tag="G")
            for b in range(B_LOC):
                sl = slice(C * b, C * (b + 1))
                nc.tensor.matmul(
                    y_ps[sl, 0:1], Whrep[sl, :], psum_[sl, :],
                    tile_position=(C * b, C * b),
                )
            ybn = smal.tile([P, 1], F32, tag="ybn")
            nc.vector.tensor_scalar(
                ybn[:], y_ps[:, 0:1], svecL[:], tvec[:], op0=MUL, op1=ADD,
            )
            yg = smal.tile([P, 1], F32, tag="yg")
            nc.scalar.activation(yg[:], ybn[:], AF.Gelu)
            o_ps = gps.tile([P, 32], F32, tag="G")
            for b in range(B_LOC):
                nc.tensor.matmul(
                    o_ps[C * b:C * b + DOUT, 0:1],
                    Wfrep[C * b:C * (b + 1), :],
                    yg[C * b:C * (b + 1), :],
                    tile_position=(C * b, C * b),
                )
            ob = smal.tile([P, 1], F32, tag="ob")
            for b in range(B_LOC):
                sl = slice(C * b, C * b + DOUT)
                nc.vector.tensor_tensor(ob[sl, :], o_ps[sl, 0:1], bf_s[sl, :], op=ADD)
            for b in range(B_LOC):
                nc.sync.dma_start(
                    out_d[b, :], ob[C * b:C * b + DOUT, 0],
                )

    _fix_sync_waits(nc)
    return nc


def _pack_x(xc):
    """[B_LOC, C, L] f32 -> [NSL, 128, NCH*129] bf16 token-major chunked
    tiles with a ones column per 128-token chunk (gram ksum extension)."""
    import ml_dtypes
    xt = np.ascontiguousarray(xc.transpose(2, 0, 1)).reshape(L, P)
    xs = xt.reshape(NSL, NCH, 128, P).transpose(0, 2, 1, 3)  # [s, t, k, col]
    out = np.empty((NSL, 128, NCH, EXT), dtype=ml_dtypes.bfloat16)
    out[..., 0:128] = xs.astype(ml_dtypes.bfloat16)
    out[..., 128] = 1.0
    return out.reshape(NSL, 128, NCH * EXT)


def _pack_params(arrs):
    """Host-side pre-replication of all weights/biases into one bf16 and one
    f32 [128, K] tensor (see build_program for the column layout)."""
    import ml_dtypes
    reps = []
    for i in range(NB):
        reps.append(np.tile(arrs["fcW1"][i], (B_LOC, 1)))          # [128,32]
    eye4 = np.eye(B_LOC, dtype=np.float32)
    for i in range(NB):
        reps.append(np.kron(eye4, arrs["fcW2"][i]))                # [128,128]
    pbf = np.concatenate(reps, axis=1).astype(ml_dtypes.bfloat16)

    cols = []
    for i in range(NB):
        cols.append(np.tile(arrs["fcb1"][i], B_LOC)[:, None])
    for i in range(NB):
        cols.append(np.tile(arrs["fcb2"][i], B_LOC)[:, None])
    cols.append(np.tile(arrs["Wh"], (B_LOC, 1)))                   # [128,32]
    cols.append(np.tile(arrs["Wf"], (B_LOC, 1)))                   # [128,10]
    for k in ("bh", "bn_gamma", "bn_beta", "bn_mean", "bn_var"):
        cols.append(np.tile(arrs[k], B_LOC)[:, None])
    bf_col = np.zeros((P, 1), np.float32)
    for b in range(B_LOC):
        bf_col[C * b:C * b + DOUT, 0] = arrs["bf"]
    cols.append(bf_col)
    pf = np.concatenate(cols, axis=1).astype(np.float32)
    assert pf.shape[1] == NPF, pf.shape
    return pbf, pf


def prep_in_maps(arrs, n_cores=8):
    x = arrs["x"]
    bl = x.shape[0] // n_cores
    pbf, pf = _pack_params(arrs)
    return [
        {"x": _pack_x(x[bl * i: bl * (i + 1)]), "pbf": pbf, "pf32": pf}
        for i in range(n_cores)
    ]


_NC_CACHE = [None]


def kernel(**inputs) -> np.ndarray:
    arrs = {k: np.asarray(v, dtype=np.float32) for k, v in inputs.items()}
    n_cores = 8

    if _NC_CACHE[0] is None:
        _NC_CACHE[0] = build_program()
    nc = _NC_CACHE[0]

    in_maps = prep_in_maps(arrs, n_cores)
    res = run_bass_kernel_spmd(nc, in_maps, list(range(n_cores))).results
    return np.concatenate([res[i]["out"] for i in range(n_cores)], axis=0)
